# revision 26
# baseline (speedup 1.0000x reference)
"""Trainium2 Bass kernel for nn_ChannelAttention (B=16, C=256, T=2048, L=5).

Data-parallel over 8 NeuronCores: each core processes 2 batches.

Math (per batch b):
  qsum[l,t]   = qws[l] @ x[:,t] + qbs[l]                      (qws = q_w.sum(1))
  scores[c,t] = sum_l (k_w[l] @ (x * Bqsum[l]))[c, t-l] + sum_l k_b[l,c]*qsum[l,t-l]
  w = softmax_c(scores);  v = PReLU(BN(v_w @ x + v_b));  out = w * v

Final design (fp16 datapath; HW 128us vs 272us fp32r baseline, rel err
1.2e-2 vs the 2e-2 budget):
  - everything feeding the PE is fp16 (1 cyc/row matmuls, FWL weight loads,
    half the DMA/SBUF traffic); PSUM accumulation stays fp32.
  - bq (lag-shifted qsum row broadcast over 128 partitions) comes straight
    from the DRAM scratch via stride-0-partition DMAs (ap=[[0,128],[1,N]]),
    pre-shifted per lag (odd lags get one extra column of shift) so BOTH
    gating operands are 4B-aligned and the DVE runs its fp16 2x mode; the
    score matmul compensates with a +1 rhs offset.  No broadcast matmuls,
    no PSUM->SBUF copies.
  - gating multiplies emitted for the whole batch up front at [128, 1024]
    double-chunk granularity (DVE 2x ~690ns; lag 4 on GpSimd) so the DVE
    never starves the PE's score matmuls behind softmax work.
  - softmax over channels in the TRANSPOSED domain: PE-transpose scores
    (fp32r, 1.5 cyc/row), per-block DVE max-reduce + fused negate, ACT Exp
    with per-partition bias=-max, one merged DVE channel-sum reduce, tiny
    DVE reciprocal [128,4], per-partition renormalize split DVE/ACT,
    fp16 PE-transpose back, output multiply vs v from PSUM at 2x.
  - v phase grouped per batch so ACT Prelu<->Exp table switches happen at
    most twice per batch (ACT_TABLE_LOAD is 1.28us each).
  - DMA descriptor counts are the startup bottleneck (128 per
    partition-strided transfer): one DMA per x half-tensor, single bql/qssh
    transfers, paired output DMAs.
"""

import sys

sys.path.insert(0, "/opt/trn_rl_repo")

import numpy as np

import concourse.bass as bass
import concourse.mybir as mybir
import concourse.tile as tile
from concourse import bacc
from concourse.bass_utils import run_bass_kernel_spmd

B, C, T, L = 16, 256, 2048, 5
NCORES = 8
BPC = B // NCORES      # batches per core
P = 128                # partitions
KC = C // P            # k chunks (2)
MC = C // P            # m chunks (2)
NT = 512               # time tile
NB = NT // P           # transpose blocks per time tile (4)
NCHUNK = T // NT       # 4
PAD = 8                # left zero pad (t<0 lag windows)
TP = PAD + T           # padded time length
QPITCH = T + 16        # dram scratch row pitch for shifted qsum
BN_EPS = 1e-5

F32 = mybir.dt.float32
F32R = mybir.dt.float32r
F16 = mybir.dt.float16

# CoreSim lacks the Prelu activation: the sim path computes
# v = max(z, alpha*z) with two DVE ops instead.
PRELU_EXPLICIT = False

AF = mybir.ActivationFunctionType
ALU = mybir.AluOpType


def build_program(alpha: float) -> bass.Bass:
    nc = bacc.Bacc("TRN2", target_bir_lowering=False, debug=False, num_devices=NCORES)

    x_in = nc.dram_tensor("x", [BPC, KC, P, TP], F16, kind="ExternalInput").ap()
    kwT_in = nc.dram_tensor("kwT", [P, L, KC, MC, P], F16, kind="ExternalInput").ap()
    kb_in = nc.dram_tensor("kb", [L, MC, P], F16, kind="ExternalInput").ap()
    qwsT_in = nc.dram_tensor("qwsT", [P, KC, L], F16, kind="ExternalInput").ap()
    qbs_in = nc.dram_tensor("qbs", [L, 1], F32, kind="ExternalInput").ap()
    vwT_in = nc.dram_tensor("vwT", [P, KC, MC, P], F16, kind="ExternalInput").ap()
    vb_in = nc.dram_tensor("vb", [P, MC], F32, kind="ExternalInput").ap()
    ident_in = nc.dram_tensor("ident", [P, P], F32R, kind="ExternalInput").ap()
    ident16_in = nc.dram_tensor("ident16", [P, P], F16, kind="ExternalInput").ap()
    y_out = nc.dram_tensor("y", [BPC, MC, P, T], F16, kind="ExternalOutput").ap()
    # scratch for the lag-shift of qsum rows (row l shifted right by l)
    qsd = nc.dram_tensor("qs_scratch", [BPC, L, QPITCH], F16).ap()

    from contextlib import ExitStack

    with tile.TileContext(nc) as tc:
        with ExitStack() as ctx:
            ep = ctx.enter_context
            ep(nc.allow_low_precision(
                reason="fp16 datapath validated at 6.4e-3 rel err vs the "
                       "2e-2 budget; PSUM accumulation stays fp32"
            ))
            consts = ep(tc.tile_pool(name="consts", bufs=1))
            xpool = ep(tc.tile_pool(name="xpool", bufs=2))
            qspool = ep(tc.tile_pool(name="qspool", bufs=2))
            qsshpool = ep(tc.tile_pool(name="qsshpool", bufs=2))
            bqlpool = ep(tc.tile_pool(name="bqlpool", bufs=10))
            vpool = ep(tc.tile_pool(name="vpool", bufs=12))
            wpool = ep(tc.tile_pool(name="wpool", bufs=12))
            spool = ep(tc.tile_pool(name="spool", bufs=4))
            epool = ep(tc.tile_pool(name="epool", bufs=6))
            accpool = ep(tc.tile_pool(name="accpool", bufs=12))
            opool = ep(tc.tile_pool(name="opool", bufs=4))
            # PSUM: 8 banks.  pscore 3 + pbq 2 (qsum & v share) + pT 3.
            pscore_pool = ep(tc.tile_pool(name="pscore", bufs=3, space="PSUM"))
            pbq_pool = ep(tc.tile_pool(name="pbq", bufs=2, space="PSUM"))
            pT_pool = ep(tc.tile_pool(name="pT", bufs=3, space="PSUM"))

            def load_x(b):
                # one DMA per kc: descriptor count is per-partition, so
                # splitting only multiplies descriptors
                tiles = [xpool.tile([P, TP], F16, tag=f"x{kc}",
                                    name=f"xb{b}k{kc}") for kc in range(KC)]
                for kc in range(KC):
                    nc.sync.dma_start(out=tiles[kc], in_=x_in[b, kc])
                return tiles

            # x for batch 0 first so its DMAs lead the queues
            x_pre = load_x(0)

            # ---- constants (small ones first; big/late-use ones last so
            # their descriptors don't delay the x load) ----
            qwsT = consts.tile([P, KC, L], F16)
            nc.sync.dma_start(out=qwsT, in_=qwsT_in)
            qbs = consts.tile([L, 1], F32)
            nc.sync.dma_start(out=qbs, in_=qbs_in)
            kb = consts.tile([L, MC, P], F16)
            nc.sync.dma_start(out=kb, in_=kb_in)
            vb = consts.tile([P, MC], F32)
            nc.sync.dma_start(out=vb, in_=vb_in)
            zpad = consts.tile([L, PAD], F16)        # zero left pad for qsd
            nc.vector.memset(zpad, 0.0)
            vwT = consts.tile([P, KC, MC, P], F16)
            nc.sync.dma_start(out=vwT, in_=vwT_in)
            kwT = consts.tile([P, L, KC, MC, P], F16)
            nc.sync.dma_start(out=kwT, in_=kwT_in)
            ident = consts.tile([P, P], F32R)        # PE transpose (fp32r scores)
            nc.sync.dma_start(out=ident, in_=ident_in)
            ident16 = consts.tile([P, P], F16)       # PE transpose (fp16 weights)
            nc.sync.dma_start(out=ident16, in_=ident16_in)


            for b in range(BPC):
                # ---- x: one tile per kc (clean 2-dim APs for DVE perf
                # modes), quarter-split so qsum starts early ----
                x_sbs = x_pre if b == 0 else load_x(b)

                # ---- qsum rows: qs[l,t] = qws[l] @ x[:,t] + qbs[l] ----
                qs_sb = qspool.tile([L, T], F16, tag="qs")
                for n in range(NCHUNK):
                    qs_ps = pbq_pool.tile([L, NT], F32, tag="pbq")
                    for kc in range(KC):
                        nc.tensor.matmul(
                            qs_ps,
                            qwsT[:, kc, :],
                            x_sbs[kc][:, PAD + n * NT:PAD + (n + 1) * NT],
                            start=(kc == 0),
                            stop=(kc == KC - 1),
                        )
                    nc.vector.tensor_scalar_add(
                        qs_sb[:, n * NT:(n + 1) * NT], qs_ps, qbs
                    )
                    # stream the rows to dram as they are produced
                    nc.sync.dma_start(
                        out=qsd[b, :, PAD + n * NT:PAD + (n + 1) * NT],
                        in_=qs_sb[:, n * NT:(n + 1) * NT],
                    )
                nc.sync.dma_start(out=qsd[b, :, 0:PAD], in_=zpad)
                nc.sync.dma_start(out=qsd[b, :, PAD + T:QPITCH], in_=zpad)

                # ---- shifted qsum views via DRAM round trip (half-split so
                # the first chunks unblock early) ----
                qssh_sb = qsshpool.tile([L, T], F16, tag="qssh")
                shifted = bass.AP(
                    tensor=qsd.tensor,
                    offset=b * L * QPITCH + PAD,
                    ap=[[QPITCH - 1, L], [1, T]],
                )
                nc.sync.dma_start(out=qssh_sb, in_=shifted)
                # bql[l][p, j] = qsum[l, j-8-l]: the lag-shifted qsum row
                # broadcast to all 128 partitions via a stride-0-partition DMA.
                # (cols j<8 read the previous row's tail; never used.)
                # odd lags get one extra column of shift baked into the
                # broadcast so both gating operands stay 4B-aligned
                bqls = []
                for l in range(L):
                    bql = bqlpool.tile([P, TP + 4], F16, tag="bql")
                    bcast = bass.AP(
                        tensor=qsd.tensor,
                        offset=(b * L + l) * QPITCH - l - (l % 2),
                        ap=[[0, P], [1, TP + 4]],
                    )
                    nc.sync.dma_start(out=bql, in_=bcast)
                    bqls.append(bql)

                # ---- v phase (grouped: one Prelu table window per batch) ----
                v_sbs = {}
                for n in range(NCHUNK):
                    t0 = n * NT
                    for mc in range(MC):
                        v_ps = pbq_pool.tile([P, NT], F32, tag="pbq")
                        for kc in range(KC):
                            nc.tensor.matmul(
                                v_ps,
                                vwT[:, kc, mc, :],
                                x_sbs[kc][:, PAD + t0:PAD + t0 + NT],
                                start=(kc == 0),
                                stop=(kc == KC - 1),
                            )
                        v_sb = vpool.tile([P, NT], F16, tag="v")
                        if PRELU_EXPLICIT:
                            vz = spool.tile([P, NT], F32, tag="vz")
                            nc.vector.tensor_scalar_add(vz, v_ps, vb[:, mc:mc + 1])
                            nc.vector.scalar_tensor_tensor(
                                v_sb, vz, float(alpha), vz, ALU.mult, ALU.max
                            )
                        else:
                            nc.scalar.activation(
                                out=v_sb, in_=v_ps, func=AF.Prelu,
                                bias=vb[:, mc:mc + 1], scale=1.0, alpha=alpha,
                            )
                        v_sbs[(n, mc)] = v_sb

                # ---- gated tiles for the whole batch, emitted up front so
                # the DVE never starves the PE's score matmuls behind
                # softmax work.  Double-chunk granularity (NT2 columns)
                # halves the op count. ----
                NT2 = 2 * NT
                w_pairs = {}
                for p_ in range(NCHUNK // 2):
                    sp = PAD + p_ * NT2
                    for l in (4, 0, 1, 2, 3):
                        # odd lags start one column early so both DVE
                        # operands stay 4B-aligned (2x mode); the score
                        # matmul compensates with a +1 rhs offset.
                        base = sp - l - (l % 2)
                        wd = NT2 + 2 * (l % 2)
                        for kc in range(KC):
                            w_sb = wpool.tile([P, NT2 + 2], F16, tag="w")
                            xa = x_sbs[kc][:, base:base + wd]
                            bq = bqls[l][:, sp:sp + wd]
                            if l == 4:
                                nc.gpsimd.tensor_mul(w_sb[:, 0:wd], xa, bq)
                            else:
                                nc.vector.tensor_mul(w_sb[:, 0:wd], xa, bq)
                            w_pairs[(p_, l, kc)] = w_sb

                o_sbs = [opool.tile([P, T], F16, tag=f"o{mc}",
                                    name=f"ob{b}m{mc}") for mc in range(MC)]

                # ---- time-chunk loop ----
                for n in range(NCHUNK):
                    t0 = n * NT
                    s0 = PAD + t0
                    p_, off = n // 2, (n % 2) * NT

                    # scores: ps[mc] = sum_{l,kc} kwT[l,kc,mc].T @ w[l,kc]
                    #         + kb[:,mc].T @ qssh[:, t0:t0+NT]
                    pscores = []
                    for mc in range(MC):
                        ps = pscore_pool.tile([P, NT], F32, tag="ps")
                        for l in range(L):
                            for kc in range(KC):
                                woff = off + l + (l % 2) - l
                                nc.tensor.matmul(
                                    ps,
                                    kwT[:, l, kc, mc, :],
                                    w_pairs[(p_, l, kc)][:, woff:woff + NT],
                                    start=(l == 0 and kc == 0),
                                    stop=False,
                                )
                        nc.tensor.matmul(
                            ps,
                            kb[:, mc, :],
                            qssh_sb[:, t0:t0 + NT],
                            start=False, stop=True,
                        )
                        pscores.append(ps)

                    # ---- softmax over channels in the transposed domain ----
                    s_sbs = []
                    sTs = []
                    for mc in range(MC):
                        s_sb = spool.tile([P, NT], F32R, tag="s")
                        nc.scalar.copy(out=s_sb, in_=pscores[mc])
                        s_sbs.append(s_sb)
                        sT = pT_pool.tile([P, NB, P], F32R, tag="pT")
                        for i in range(NB):
                            nc.tensor.transpose(
                                sT[:, i, :], s_sb[:, i * P:(i + 1) * P], ident
                            )
                        sTs.append(sT.bitcast(F32))
                    # per-block maxes of both halves land in one [P, NB, 2]
                    # tile; a single negated reduce over the last axis then
                    # yields nmax[p,i] = -max(all 256 channels).  (Per-block
                    # biases: every block sum contains e^0 = 1, so the fp16
                    # eT tiles can never underflow to an all-zero row.)
                    maxT2 = accpool.tile([P, NB, 2], F32, tag="maxT2")
                    for mc in range(MC):
                        nc.vector.tensor_reduce(
                            out=maxT2[:, :, mc:mc + 1], in_=sTs[mc],
                            axis=mybir.AxisListType.X, op=ALU.max,
                        )
                    nmax = accpool.tile([P, NB], F32, tag="nmax")
                    nc.vector.tensor_reduce(
                        out=nmax, in_=maxT2, axis=mybir.AxisListType.X,
                        op=ALU.max, negate=True,
                    )
                    # e = exp(sT - max); both halves share one tile so the
                    # channel sums take a single DVE reduce
                    eT2 = epool.tile([P, MC, NB, P], F16, tag="eT2")
                    eTs = [eT2[:, mc] for mc in range(MC)]
                    for mc in range(MC):
                        for i in range(NB):
                            nc.scalar.activation(
                                out=eT2[:, mc, i, :], in_=sTs[mc][:, i, :],
                                func=AF.Exp, bias=nmax[:, i:i + 1], scale=1.0,
                            )
                    acc2 = accpool.tile([P, MC, NB], F32, tag="acc2")
                    nc.vector.tensor_reduce(
                        out=acc2, in_=eT2, axis=mybir.AxisListType.X,
                        op=ALU.add,
                    )
                    sums = accpool.tile([P, NB], F32, tag="sums")
                    nc.vector.tensor_add(sums, acc2[:, 0], acc2[:, 1])
                    rT = accpool.tile([P, NB], F32, tag="rT")
                    nc.vector.reciprocal(rT, sums)
                    # renormalize + transpose back to channel-major
                    for mc in range(MC):
                        wT = epool.tile([P, NB, P], F16, tag="wT")
                        for i in range(NB):
                            if mc == 0:
                                nc.vector.tensor_scalar_mul(
                                    wT[:, i, :], eTs[mc][:, i, :],
                                    rT[:, i:i + 1],
                                )
                            else:
                                nc.scalar.activation(
                                    out=wT[:, i, :], in_=eTs[mc][:, i, :],
                                    func=AF.Copy, scale=rT[:, i:i + 1],
                                )
                        wb_ps = pT_pool.tile([P, NB, P], F16, tag="pT")
                        for i in range(NB):
                            nc.tensor.transpose(
                                wb_ps[:, i, :], wT[:, i, :], ident16
                            )
                        nc.vector.tensor_mul(
                            o_sbs[mc][:, t0:t0 + NT], wb_ps, v_sbs[(n, mc)]
                        )
                        if n % 2 == 1:
                            tp0 = (n - 1) * NT
                            nc.sync.dma_start(
                                out=y_out[b, mc, :, tp0:tp0 + 2 * NT],
                                in_=o_sbs[mc][:, tp0:tp0 + 2 * NT],
                            )
    nc.compile()
    return nc


def fold_weights(inputs: dict) -> dict:
    """Host-side folding of the tiny weight tensors into device layouts."""
    k_w = np.asarray(inputs["k_w"], np.float32)
    k_b = np.asarray(inputs["k_b"], np.float32)
    q_w = np.asarray(inputs["q_w"], np.float32)
    q_b = np.asarray(inputs["q_b"], np.float32)
    v_w = np.asarray(inputs["v_w"], np.float32)
    v_b = np.asarray(inputs["v_b"], np.float32)
    gamma = np.asarray(inputs["bn_gamma"], np.float32)
    beta = np.asarray(inputs["bn_beta"], np.float32)
    mean = np.asarray(inputs["bn_mean"], np.float32)
    var = np.asarray(inputs["bn_var"], np.float32)

    # kwT[p, l, kc, mc, m] = k_w[l, mc*128+m, kc*128+p]
    kwT = np.ascontiguousarray(
        k_w.reshape(L, MC, P, KC, P).transpose(4, 0, 3, 1, 2)
    ).astype(np.float16)
    kb = np.ascontiguousarray(k_b.reshape(L, MC, P)).astype(np.float16)
    qws = q_w.sum(axis=1)                       # [L, C]
    qwsT = np.ascontiguousarray(
        qws.reshape(L, KC, P).transpose(2, 1, 0)
    ).astype(np.float16)
    qbs = np.ascontiguousarray(q_b.sum(axis=1).reshape(L, 1))
    scale = gamma / np.sqrt(var + BN_EPS)
    vw_f = v_w * scale[:, None]
    vb_f = (v_b - mean) * scale + beta
    vwT = np.ascontiguousarray(
        vw_f.reshape(MC, P, KC, P).transpose(3, 2, 0, 1)
    ).astype(np.float16)
    vbT = np.ascontiguousarray(vb_f.reshape(MC, P).transpose(1, 0))
    return {
        "kwT": kwT, "kb": kb, "qwsT": qwsT, "qbs": qbs,
        "vwT": vwT, "vb": vbT,
        "ident": np.eye(P, dtype=np.float32),
        "ident16": np.eye(P, dtype=np.float16),
    }


_CACHE: dict = {}


def make_in_maps(inputs: dict) -> list:
    weights = fold_weights(inputs)
    x = np.asarray(inputs["x"], np.float32)
    # pad x on the left with zeros and convert to fp16
    xp = np.zeros((B, C, TP), np.float16)
    xp[:, :, PAD:] = x.astype(np.float16)
    xp = xp.reshape(B // BPC, BPC, KC, P, TP)
    return [
        {"x": np.ascontiguousarray(xp[i]), **weights} for i in range(NCORES)
    ]


def kernel(**inputs) -> np.ndarray:
    alpha = float(np.asarray(inputs["prelu_alpha"]).reshape(-1)[0])

    key = ("prog", alpha, PRELU_EXPLICIT)
    if key not in _CACHE:
        _CACHE[key] = build_program(alpha)
    nc = _CACHE[key]

    in_maps = make_in_maps(inputs)
    res = run_bass_kernel_spmd(nc, in_maps, list(range(NCORES)))
    outs = [r["y"].reshape(BPC, C, T).astype(np.float32) for r in res.results]
    return np.concatenate(outs, axis=0)


if __name__ == "__main__":
    rng = np.random.default_rng(0)
    demo = {
        "x": rng.standard_normal((B, C, T), dtype=np.float32),
        "q_w": rng.standard_normal((L, C, C), dtype=np.float32) / 16,
        "q_b": rng.standard_normal((L, C), dtype=np.float32) * 0.02,
        "k_w": rng.standard_normal((L, C, C), dtype=np.float32) / 16,
        "k_b": rng.standard_normal((L, C), dtype=np.float32) * 0.02,
        "v_w": rng.standard_normal((C, C), dtype=np.float32) / 16,
        "v_b": rng.standard_normal((C,), dtype=np.float32) * 0.02,
        "bn_gamma": rng.uniform(0.5, 1.5, C).astype(np.float32),
        "bn_beta": rng.standard_normal(C).astype(np.float32) * 0.02,
        "bn_mean": rng.standard_normal(C).astype(np.float32) * 0.1,
        "bn_var": rng.uniform(0.5, 1.5, C).astype(np.float32),
        "prelu_alpha": np.full((1,), 0.25, np.float32),
    }
    y = kernel(**demo)
    print("out", y.shape, y.dtype, float(np.abs(y).max()))


# revision 27
# speedup vs baseline: 1.0430x; 1.0430x over previous
"""Trainium2 Bass kernel for nn_ChannelAttention (B=16, C=256, T=2048, L=5).

Data-parallel over 8 NeuronCores: each core processes 2 batches.

Math (per batch b):
  qsum[l,t]   = qws[l] @ x[:,t] + qbs[l]                      (qws = q_w.sum(1))
  scores[c,t] = sum_l (k_w[l] @ (x * Bqsum[l]))[c, t-l] + sum_l k_b[l,c]*qsum[l,t-l]
  w = softmax_c(scores);  v = PReLU(BN(v_w @ x + v_b));  out = w * v

Final design (fp16 datapath; HW 128us vs 272us fp32r baseline, rel err
1.2e-2 vs the 2e-2 budget):
  - everything feeding the PE is fp16 (1 cyc/row matmuls, FWL weight loads,
    half the DMA/SBUF traffic); PSUM accumulation stays fp32.
  - bq (lag-shifted qsum row broadcast over 128 partitions) comes straight
    from the DRAM scratch via stride-0-partition DMAs (ap=[[0,128],[1,N]]),
    pre-shifted per lag (odd lags get one extra column of shift) so BOTH
    gating operands are 4B-aligned and the DVE runs its fp16 2x mode; the
    score matmul compensates with a +1 rhs offset.  No broadcast matmuls,
    no PSUM->SBUF copies.
  - gating multiplies emitted for the whole batch up front at [128, 1024]
    double-chunk granularity (DVE 2x ~690ns; lag 4 on GpSimd) so the DVE
    never starves the PE's score matmuls behind softmax work.
  - softmax over channels in the TRANSPOSED domain: PE-transpose scores
    (fp32r, 1.5 cyc/row), per-block DVE max-reduce + fused negate, ACT Exp
    with per-partition bias=-max, one merged DVE channel-sum reduce, tiny
    DVE reciprocal [128,4], per-partition renormalize split DVE/ACT,
    fp16 PE-transpose back, output multiply vs v from PSUM at 2x.
  - v phase grouped per batch so ACT Prelu<->Exp table switches happen at
    most twice per batch (ACT_TABLE_LOAD is 1.28us each).
  - DMA descriptor counts are the startup bottleneck (128 per
    partition-strided transfer): one DMA per x half-tensor, single bql/qssh
    transfers, paired output DMAs.
"""

import sys

sys.path.insert(0, "/opt/trn_rl_repo")

import numpy as np

import concourse.bass as bass
import concourse.mybir as mybir
import concourse.tile as tile
from concourse import bacc
from concourse.bass_utils import run_bass_kernel_spmd

B, C, T, L = 16, 256, 2048, 5
NCORES = 8
BPC = B // NCORES      # batches per core
P = 128                # partitions
KC = C // P            # k chunks (2)
MC = C // P            # m chunks (2)
NT = 512               # time tile
NB = NT // P           # transpose blocks per time tile (4)
NCHUNK = T // NT       # 4
PAD = 8                # left zero pad (t<0 lag windows)
TP = PAD + T           # padded time length
QPITCH = T + 16        # dram scratch row pitch for shifted qsum
BN_EPS = 1e-5

F32 = mybir.dt.float32
F32R = mybir.dt.float32r
F16 = mybir.dt.float16

# CoreSim lacks the Prelu activation: the sim path computes
# v = max(z, alpha*z) with two DVE ops instead.
PRELU_EXPLICIT = False

AF = mybir.ActivationFunctionType
ALU = mybir.AluOpType


def build_program(alpha: float) -> bass.Bass:
    nc = bacc.Bacc("TRN2", target_bir_lowering=False, debug=False, num_devices=NCORES)

    x_in = nc.dram_tensor("x", [BPC, KC, P, TP], F16, kind="ExternalInput").ap()
    kwT_in = nc.dram_tensor("kwT", [P, L, KC, MC, P], F16, kind="ExternalInput").ap()
    kb_in = nc.dram_tensor("kb", [L, MC, P], F16, kind="ExternalInput").ap()
    qwsT_in = nc.dram_tensor("qwsT", [P, KC, L], F16, kind="ExternalInput").ap()
    qbs_in = nc.dram_tensor("qbs", [L, 1], F32, kind="ExternalInput").ap()
    vwT_in = nc.dram_tensor("vwT", [P, KC, MC, P], F16, kind="ExternalInput").ap()
    vb_in = nc.dram_tensor("vb", [P, MC], F32, kind="ExternalInput").ap()
    ident_in = nc.dram_tensor("ident", [P, P], F32R, kind="ExternalInput").ap()
    ident16_in = nc.dram_tensor("ident16", [P, P], F16, kind="ExternalInput").ap()
    y_out = nc.dram_tensor("y", [BPC, MC, P, T], F16, kind="ExternalOutput").ap()
    # scratch for the lag-shift of qsum rows (row l shifted right by l)
    qsd = nc.dram_tensor("qs_scratch", [BPC, L, QPITCH], F16).ap()

    from contextlib import ExitStack

    with tile.TileContext(nc) as tc:
        with ExitStack() as ctx:
            ep = ctx.enter_context
            ep(nc.allow_low_precision(
                reason="fp16 datapath validated at 6.4e-3 rel err vs the "
                       "2e-2 budget; PSUM accumulation stays fp32"
            ))
            consts = ep(tc.tile_pool(name="consts", bufs=1))
            xpool = ep(tc.tile_pool(name="xpool", bufs=2))
            qspool = ep(tc.tile_pool(name="qspool", bufs=2))
            qsshpool = ep(tc.tile_pool(name="qsshpool", bufs=2))
            bqlpool = ep(tc.tile_pool(name="bqlpool", bufs=10))
            vpool = ep(tc.tile_pool(name="vpool", bufs=12))
            wpool = ep(tc.tile_pool(name="wpool", bufs=12))
            spool = ep(tc.tile_pool(name="spool", bufs=4))
            epool = ep(tc.tile_pool(name="epool", bufs=6))
            accpool = ep(tc.tile_pool(name="accpool", bufs=12))
            opool = ep(tc.tile_pool(name="opool", bufs=4))
            # PSUM: 8 banks.  pscore 3 + pbq 2 (qsum & v share) + pT 3.
            pscore_pool = ep(tc.tile_pool(name="pscore", bufs=3, space="PSUM"))
            pbq_pool = ep(tc.tile_pool(name="pbq", bufs=2, space="PSUM"))
            pT_pool = ep(tc.tile_pool(name="pT", bufs=3, space="PSUM"))

            def load_x(b):
                # one DMA per kc: descriptor count is per-partition, so
                # splitting only multiplies descriptors
                tiles = [xpool.tile([P, TP], F16, tag=f"x{kc}",
                                    name=f"xb{b}k{kc}") for kc in range(KC)]
                for kc in range(KC):
                    nc.sync.dma_start(out=tiles[kc], in_=x_in[b, kc])
                return tiles

            # x for batch 0 first so its DMAs lead the queues
            x_pre = load_x(0)

            # ---- constants (small ones first; big/late-use ones last so
            # their descriptors don't delay the x load) ----
            qwsT = consts.tile([P, KC, L], F16)
            nc.sync.dma_start(out=qwsT, in_=qwsT_in)
            qbs = consts.tile([L, 1], F32)
            nc.sync.dma_start(out=qbs, in_=qbs_in)
            kb = consts.tile([L, MC, P], F16)
            nc.sync.dma_start(out=kb, in_=kb_in)
            vb = consts.tile([P, MC], F32)
            nc.sync.dma_start(out=vb, in_=vb_in)
            zpad = consts.tile([L, PAD], F16)        # zero left pad for qsd
            nc.vector.memset(zpad, 0.0)
            vwT = consts.tile([P, KC, MC, P], F16)
            nc.sync.dma_start(out=vwT, in_=vwT_in)
            kwT = consts.tile([P, L, KC, MC, P], F16)
            nc.sync.dma_start(out=kwT, in_=kwT_in)
            ident = consts.tile([P, P], F32R)        # PE transpose (fp32r scores)
            nc.sync.dma_start(out=ident, in_=ident_in)
            ident16 = consts.tile([P, P], F16)       # PE transpose (fp16 weights)
            nc.sync.dma_start(out=ident16, in_=ident16_in)


            for b in range(BPC):
                # ---- x: one tile per kc (clean 2-dim APs for DVE perf
                # modes), quarter-split so qsum starts early ----
                x_sbs = x_pre if b == 0 else load_x(b)

                # zero the scratch row pads first so the shifted reads
                # below only wait on the qsum row DMAs
                nc.sync.dma_start(out=qsd[b, :, 0:PAD], in_=zpad)
                nc.sync.dma_start(out=qsd[b, :, PAD + T:QPITCH], in_=zpad)

                # ---- qsum rows: qs[l,t] = qws[l] @ x[:,t] + qbs[l] ----
                qs_sb = qspool.tile([L, T], F16, tag="qs")
                for n in range(NCHUNK):
                    qs_ps = pbq_pool.tile([L, NT], F32, tag="pbq")
                    for kc in range(KC):
                        nc.tensor.matmul(
                            qs_ps,
                            qwsT[:, kc, :],
                            x_sbs[kc][:, PAD + n * NT:PAD + (n + 1) * NT],
                            start=(kc == 0),
                            stop=(kc == KC - 1),
                        )
                    nc.vector.tensor_scalar_add(
                        qs_sb[:, n * NT:(n + 1) * NT], qs_ps, qbs
                    )
                    # stream the rows to dram as they are produced
                    nc.sync.dma_start(
                        out=qsd[b, :, PAD + n * NT:PAD + (n + 1) * NT],
                        in_=qs_sb[:, n * NT:(n + 1) * NT],
                    )
                nc.sync.dma_start(out=qsd[b, :, 0:PAD], in_=zpad)
                nc.sync.dma_start(out=qsd[b, :, PAD + T:QPITCH], in_=zpad)

                # ---- shifted qsum views via DRAM round trip (half-split so
                # the first chunks unblock early) ----
                qssh_sb = qsshpool.tile([L, T], F16, tag="qssh")
                shifted = bass.AP(
                    tensor=qsd.tensor,
                    offset=b * L * QPITCH + PAD,
                    ap=[[QPITCH - 1, L], [1, T]],
                )
                nc.sync.dma_start(out=qssh_sb, in_=shifted)
                # bql[l][p, j] = qsum[l, j-8-l]: the lag-shifted qsum row
                # broadcast to all 128 partitions via a stride-0-partition DMA.
                # (cols j<8 read the previous row's tail; never used.)
                # odd lags get one extra column of shift baked into the
                # broadcast so both gating operands stay 4B-aligned
                bqls = []
                for l in range(L):
                    bql = bqlpool.tile([P, TP + 4], F16, tag="bql")
                    bcast = bass.AP(
                        tensor=qsd.tensor,
                        offset=(b * L + l) * QPITCH - l - (l % 2),
                        ap=[[0, P], [1, TP + 4]],
                    )
                    nc.sync.dma_start(out=bql, in_=bcast)
                    bqls.append(bql)

                # ---- v phase (grouped: one Prelu table window per batch) ----
                v_sbs = {}
                for n in range(NCHUNK):
                    t0 = n * NT
                    for mc in range(MC):
                        v_ps = pbq_pool.tile([P, NT], F32, tag="pbq")
                        for kc in range(KC):
                            nc.tensor.matmul(
                                v_ps,
                                vwT[:, kc, mc, :],
                                x_sbs[kc][:, PAD + t0:PAD + t0 + NT],
                                start=(kc == 0),
                                stop=(kc == KC - 1),
                            )
                        v_sb = vpool.tile([P, NT], F16, tag="v")
                        if PRELU_EXPLICIT:
                            vz = spool.tile([P, NT], F32, tag="vz")
                            nc.vector.tensor_scalar_add(vz, v_ps, vb[:, mc:mc + 1])
                            nc.vector.scalar_tensor_tensor(
                                v_sb, vz, float(alpha), vz, ALU.mult, ALU.max
                            )
                        else:
                            nc.scalar.activation(
                                out=v_sb, in_=v_ps, func=AF.Prelu,
                                bias=vb[:, mc:mc + 1], scale=1.0, alpha=alpha,
                            )
                        v_sbs[(n, mc)] = v_sb

                # ---- gated tiles for the whole batch, emitted up front so
                # the DVE never starves the PE's score matmuls behind
                # softmax work.  Double-chunk granularity (NT2 columns)
                # halves the op count. ----
                NT2 = 2 * NT
                w_pairs = {}
                for p_ in range(NCHUNK // 2):
                    sp = PAD + p_ * NT2
                    for l in (4, 0, 1, 2, 3):
                        # odd lags start one column early so both DVE
                        # operands stay 4B-aligned (2x mode); the score
                        # matmul compensates with a +1 rhs offset.
                        base = sp - l - (l % 2)
                        wd = NT2 + 2 * (l % 2)
                        for kc in range(KC):
                            w_sb = wpool.tile([P, NT2 + 2], F16, tag="w")
                            xa = x_sbs[kc][:, base:base + wd]
                            bq = bqls[l][:, sp:sp + wd]
                            if l == 4 and kc == 1:
                                nc.gpsimd.tensor_mul(w_sb[:, 0:wd], xa, bq)
                            else:
                                nc.vector.tensor_mul(w_sb[:, 0:wd], xa, bq)
                            w_pairs[(p_, l, kc)] = w_sb

                o_sbs = [opool.tile([P, T], F16, tag=f"o{mc}",
                                    name=f"ob{b}m{mc}") for mc in range(MC)]

                # ---- time-chunk loop ----
                for n in range(NCHUNK):
                    t0 = n * NT
                    s0 = PAD + t0
                    p_, off = n // 2, (n % 2) * NT

                    # scores: ps[mc] = sum_{l,kc} kwT[l,kc,mc].T @ w[l,kc]
                    #         + kb[:,mc].T @ qssh[:, t0:t0+NT]
                    pscores = []
                    for mc in range(MC):
                        ps = pscore_pool.tile([P, NT], F32, tag="ps")
                        for l in range(L):
                            for kc in range(KC):
                                woff = off + l + (l % 2) - l
                                nc.tensor.matmul(
                                    ps,
                                    kwT[:, l, kc, mc, :],
                                    w_pairs[(p_, l, kc)][:, woff:woff + NT],
                                    start=(l == 0 and kc == 0),
                                    stop=False,
                                )
                        nc.tensor.matmul(
                            ps,
                            kb[:, mc, :],
                            qssh_sb[:, t0:t0 + NT],
                            start=False, stop=True,
                        )
                        pscores.append(ps)

                    # ---- softmax over channels in the transposed domain ----
                    s_sbs = []
                    sTs = []
                    for mc in range(MC):
                        s_sb = spool.tile([P, NT], F32R, tag="s")
                        nc.scalar.copy(out=s_sb, in_=pscores[mc])
                        s_sbs.append(s_sb)
                        sT = pT_pool.tile([P, NB, P], F32R, tag="pT")
                        for i in range(NB):
                            nc.tensor.transpose(
                                sT[:, i, :], s_sb[:, i * P:(i + 1) * P], ident
                            )
                        sTs.append(sT.bitcast(F32))
                    # per-block maxes of both halves land in one [P, NB, 2]
                    # tile; a single negated reduce over the last axis then
                    # yields nmax[p,i] = -max(all 256 channels).  (Per-block
                    # biases: every block sum contains e^0 = 1, so the fp16
                    # eT tiles can never underflow to an all-zero row.)
                    maxT2 = accpool.tile([P, NB, 2], F32, tag="maxT2")
                    for mc in range(MC):
                        nc.vector.tensor_reduce(
                            out=maxT2[:, :, mc:mc + 1], in_=sTs[mc],
                            axis=mybir.AxisListType.X, op=ALU.max,
                        )
                    nmax = accpool.tile([P, NB], F32, tag="nmax")
                    nc.vector.tensor_reduce(
                        out=nmax, in_=maxT2, axis=mybir.AxisListType.X,
                        op=ALU.max, negate=True,
                    )
                    # e = exp(sT - max); both halves share one tile so the
                    # channel sums take a single DVE reduce
                    eT2 = epool.tile([P, MC, NB, P], F16, tag="eT2")
                    eTs = [eT2[:, mc] for mc in range(MC)]
                    for mc in range(MC):
                        for i in range(NB):
                            nc.scalar.activation(
                                out=eT2[:, mc, i, :], in_=sTs[mc][:, i, :],
                                func=AF.Exp, bias=nmax[:, i:i + 1], scale=1.0,
                            )
                    acc2 = accpool.tile([P, MC, NB], F32, tag="acc2")
                    nc.vector.tensor_reduce(
                        out=acc2, in_=eT2, axis=mybir.AxisListType.X,
                        op=ALU.add,
                    )
                    sums = accpool.tile([P, NB], F32, tag="sums")
                    nc.vector.tensor_add(sums, acc2[:, 0], acc2[:, 1])
                    rT = accpool.tile([P, NB], F32, tag="rT")
                    nc.vector.reciprocal(rT, sums)
                    # renormalize + transpose back to channel-major
                    for mc in range(MC):
                        wT = epool.tile([P, NB, P], F16, tag="wT")
                        for i in range(NB):
                            if mc == 0:
                                nc.vector.tensor_scalar_mul(
                                    wT[:, i, :], eTs[mc][:, i, :],
                                    rT[:, i:i + 1],
                                )
                            else:
                                nc.scalar.activation(
                                    out=wT[:, i, :], in_=eTs[mc][:, i, :],
                                    func=AF.Copy, scale=rT[:, i:i + 1],
                                )
                        wb_ps = pT_pool.tile([P, NB, P], F16, tag="pT")
                        for i in range(NB):
                            nc.tensor.transpose(
                                wb_ps[:, i, :], wT[:, i, :], ident16
                            )
                        nc.vector.tensor_mul(
                            o_sbs[mc][:, t0:t0 + NT], wb_ps, v_sbs[(n, mc)]
                        )
                        if n % 2 == 1:
                            tp0 = (n - 1) * NT
                            nc.sync.dma_start(
                                out=y_out[b, mc, :, tp0:tp0 + 2 * NT],
                                in_=o_sbs[mc][:, tp0:tp0 + 2 * NT],
                            )
    nc.compile()
    return nc


def fold_weights(inputs: dict) -> dict:
    """Host-side folding of the tiny weight tensors into device layouts."""
    k_w = np.asarray(inputs["k_w"], np.float32)
    k_b = np.asarray(inputs["k_b"], np.float32)
    q_w = np.asarray(inputs["q_w"], np.float32)
    q_b = np.asarray(inputs["q_b"], np.float32)
    v_w = np.asarray(inputs["v_w"], np.float32)
    v_b = np.asarray(inputs["v_b"], np.float32)
    gamma = np.asarray(inputs["bn_gamma"], np.float32)
    beta = np.asarray(inputs["bn_beta"], np.float32)
    mean = np.asarray(inputs["bn_mean"], np.float32)
    var = np.asarray(inputs["bn_var"], np.float32)

    # kwT[p, l, kc, mc, m] = k_w[l, mc*128+m, kc*128+p]
    kwT = np.ascontiguousarray(
        k_w.reshape(L, MC, P, KC, P).transpose(4, 0, 3, 1, 2)
    ).astype(np.float16)
    kb = np.ascontiguousarray(k_b.reshape(L, MC, P)).astype(np.float16)
    qws = q_w.sum(axis=1)                       # [L, C]
    qwsT = np.ascontiguousarray(
        qws.reshape(L, KC, P).transpose(2, 1, 0)
    ).astype(np.float16)
    qbs = np.ascontiguousarray(q_b.sum(axis=1).reshape(L, 1))
    scale = gamma / np.sqrt(var + BN_EPS)
    vw_f = v_w * scale[:, None]
    vb_f = (v_b - mean) * scale + beta
    vwT = np.ascontiguousarray(
        vw_f.reshape(MC, P, KC, P).transpose(3, 2, 0, 1)
    ).astype(np.float16)
    vbT = np.ascontiguousarray(vb_f.reshape(MC, P).transpose(1, 0))
    return {
        "kwT": kwT, "kb": kb, "qwsT": qwsT, "qbs": qbs,
        "vwT": vwT, "vb": vbT,
        "ident": np.eye(P, dtype=np.float32),
        "ident16": np.eye(P, dtype=np.float16),
    }


_CACHE: dict = {}


def make_in_maps(inputs: dict) -> list:
    weights = fold_weights(inputs)
    x = np.asarray(inputs["x"], np.float32)
    # pad x on the left with zeros and convert to fp16
    xp = np.zeros((B, C, TP), np.float16)
    xp[:, :, PAD:] = x.astype(np.float16)
    xp = xp.reshape(B // BPC, BPC, KC, P, TP)
    return [
        {"x": np.ascontiguousarray(xp[i]), **weights} for i in range(NCORES)
    ]


def kernel(**inputs) -> np.ndarray:
    alpha = float(np.asarray(inputs["prelu_alpha"]).reshape(-1)[0])

    key = ("prog", alpha, PRELU_EXPLICIT)
    if key not in _CACHE:
        _CACHE[key] = build_program(alpha)
    nc = _CACHE[key]

    in_maps = make_in_maps(inputs)
    res = run_bass_kernel_spmd(nc, in_maps, list(range(NCORES)))
    outs = [r["y"].reshape(BPC, C, T).astype(np.float32) for r in res.results]
    return np.concatenate(outs, axis=0)


if __name__ == "__main__":
    rng = np.random.default_rng(0)
    demo = {
        "x": rng.standard_normal((B, C, T), dtype=np.float32),
        "q_w": rng.standard_normal((L, C, C), dtype=np.float32) / 16,
        "q_b": rng.standard_normal((L, C), dtype=np.float32) * 0.02,
        "k_w": rng.standard_normal((L, C, C), dtype=np.float32) / 16,
        "k_b": rng.standard_normal((L, C), dtype=np.float32) * 0.02,
        "v_w": rng.standard_normal((C, C), dtype=np.float32) / 16,
        "v_b": rng.standard_normal((C,), dtype=np.float32) * 0.02,
        "bn_gamma": rng.uniform(0.5, 1.5, C).astype(np.float32),
        "bn_beta": rng.standard_normal(C).astype(np.float32) * 0.02,
        "bn_mean": rng.standard_normal(C).astype(np.float32) * 0.1,
        "bn_var": rng.uniform(0.5, 1.5, C).astype(np.float32),
        "prelu_alpha": np.full((1,), 0.25, np.float32),
    }
    y = kernel(**demo)
    print("out", y.shape, y.dtype, float(np.abs(y).max()))
